# revision 42
# baseline (speedup 1.0000x reference)
"""ParticleFlowNetwork TRN2 Bass kernel (v2 — software-pipelined).

Network (B=4096, P=128, IN=4, H=100):
    h = x @ W0 + b0            (no ReLU)
    h = relu(h @ W1 + b1); h = relu(h @ W2 + b2); h = h @ W3 + b3
    lat = sum over P; 5-layer event head; softmax(2)

Host-side algebraic folds (linear, exact):
  * W01 = W0 @ W1, b1' = b0 @ W1 + b1      (no ReLU after layer 0)
  * pooling moved before W3; W3U0 = W3 @ U0, c0' = P*(b3 @ U0) + c0
  * 2-class softmax == sigmoid of +/- logit difference.

Device structure per core (512 batch rows, 4 supertiles x 128 rows):
  * x pre-cast to bf16 on host. Supertile 0 transposed via PE matmul
    (x tile stationary x identity) while the weight DMAs stream;
    supertiles 1-3 via DMA xbar transpose (DRAM->SBUF, off PE).
  * all activations transposed [hidden, tokens]; bf16 matmul operands,
    fp32 PSUM. Per supertile, 32 particle-tiles t of [100, 512].
  * 5-stage software pipeline over 64 token-pairs (L01 pair -> ep1 ->
    L2 pair -> ep2 x2 -> pool pair, with 1-iteration offsets) so PE
    matmuls stream back-to-back at ~215 ns/512-col.
  * engine DEDICATION avoids FIFO head-of-line blocking: ACT runs all
    2t-wide ep1 (bias+ReLU, PSUM->SBUF, [100,1024]); DVE runs the 1t
    ep2s except every 6th pair's first half on ACT for load balance.
  * pooling = PSUM-accumulated matmuls (W3U0 as lhsT) over all 32
    particle tiles; the 4 column-block partial sums are folded by
    GpSimd (DVE for the last supertile) after an ACT PSUM->SBUF copy.
  * event head pipelined in column halves; softmax via one Sigmoid on
    [2, 512]; output stays [2, 512] per core (host reassembles).

Measured on 8 axon trn2 cores: ~111.2 us NEFF exec, rel err 1.4e-4.
PSUM budget: h1 [100,1024]x2 + h2 [100,512]x3 + z0 [100,512]x1 = 8 banks.
"""

from contextlib import ExitStack

import numpy as np
import ml_dtypes

import concourse.bass as bass
import concourse.tile as tile
from concourse import bacc, mybir
from concourse._compat import with_exitstack
from concourse.bass_utils import run_bass_kernel_spmd

F32 = mybir.dt.float32
BF16 = mybir.dt.bfloat16
AF = mybir.ActivationFunctionType
ALU = mybir.AluOpType

B, P, IN, H = 4096, 128, 4, 100
NCORES = 8
BS = B // NCORES          # 512 batch rows per core
SUP = 4                   # supertiles per core
SB = BS // SUP            # 128 batch rows per supertile
ROW = P * IN              # 512 bf16 per batch row
NT = P // 4               # 32 particle-tiles (t) per supertile
PAIRS_PER_SUP = NT // 2   # 16
PAIRS = SUP * PAIRS_PER_SUP  # 64 global pair iterations
W01_CHUNK = 4             # t-slices per w01 DMA chunk


@with_exitstack
def _body(ctx: ExitStack, tc: "tile.TileContext", d):
    nc = tc.nc

    const = ctx.enter_context(tc.tile_pool(name="const", bufs=1))
    xt_pool = ctx.enter_context(tc.tile_pool(name="xt", bufs=2))
    h1s_pool = ctx.enter_context(tc.tile_pool(name="h1s", bufs=3))
    h2s_pool = ctx.enter_context(tc.tile_pool(name="h2s", bufs=5))
    z_pool = ctx.enter_context(tc.tile_pool(name="z", bufs=1))
    ps_h1 = ctx.enter_context(tc.tile_pool(name="ps_h1", bufs=2, space="PSUM"))
    ps_h2 = ctx.enter_context(tc.tile_pool(name="ps_h2", bufs=3, space="PSUM"))
    ps_z = ctx.enter_context(tc.tile_pool(name="ps_z", bufs=1, space="PSUM"))

    # pipeline state (xt declared before first transpose emission)
    xt_sb = [None] * SUP

    def emit_transposes(s, split=False):
        # x rows 128s..128s+128, bf16 [128, ROW] -> xT [128, ROW] via xbar
        xt_sb[s] = xt_pool.tile([128, ROW], BF16, tag="xt_sb", name=f"xt_sb{s}")
        for m in range(4):
            eng = nc.scalar if (split and m % 2) else nc.sync
            eng.dma_start_transpose(
                xt_sb[s][:, 128 * m:128 * (m + 1)],
                d["x"].ap()[s * SB:(s + 1) * SB, 128 * m:128 * (m + 1)],
            )

    # supertile 0 via PE transpose (PE idle at startup; xbar path would
    # serialize against the startup weight DMAs on the HWDGE engines).
    # ident first, then x in quarters so each lands on its own DMA queue
    # and the first transpose can start ~2us earlier.
    x_sb = const.tile([128, ROW], BF16, tag="x_sb")
    for hm in range(2):
        nc.sync.dma_start(x_sb[:, 256 * hm:256 * (hm + 1)],
                          d["x"].ap()[0:SB, 256 * hm:256 * (hm + 1)])
    ident = const.tile([128, 128], BF16, tag="ident")
    nc.sync.dma_start(ident[:], d["ident"].ap())
    bias = const.tile([128, 8], F32, tag="bias")
    nc.scalar.dma_start(bias[:], d["bias"].ap())
    b1p, b2 = bias[:H, 0:1], bias[:H, 1:2]
    c0p, c1, c2, c3 = bias[:H, 2:3], bias[:H, 3:4], bias[:H, 4:5], bias[:H, 5:6]
    c4d = bias[0:2, 6:7]

    # dummy sigmoid so walrus loads the sigmoid table-set (contains relu
    # too) once at startup instead of a second ACT_TABLE_LOAD at the tail
    dummy = const.tile([1, 1], F32, tag="dummy")
    nc.scalar.activation(dummy[:], bias[0:1, 7:8], AF.Sigmoid)

    w01c = []
    w01_dmas = []
    for j in range(NT // W01_CHUNK):
        w = const.tile([128, W01_CHUNK * H], BF16, tag=f"w01_{j}", name=f"w01_{j}")
        w01c.append(w)

    def emit_w01(j, eng=None):
        (eng or nc.sync).dma_start(
            w01c[j][:],
            d["w01"].ap()[:, j * W01_CHUNK * H:(j + 1) * W01_CHUNK * H])

    def w01_slice(t):
        return w01c[t // W01_CHUNK][:, (t % W01_CHUNK) * H:(t % W01_CHUNK + 1) * H]

    emit_w01(0, eng=nc.scalar)
    consts = {}
    for name, shape in (("w2", [H, H]), ("w3u0", [H, H])):
        tl = const.tile(shape, BF16, tag=name, name=f"c_{name}")
        nc.sync.dma_start(tl[:], d[name].ap())
        consts[name] = tl
    for j in range(1, NT // W01_CHUNK):
        emit_w01(j)
    # head weights are only needed at the very end; declared here, DMA'd
    # after the main loop so they don't clog the startup DMA queue
    for name, shape in (("u1", [H, H]), ("u2", [H, H]), ("u3", [H, H]),
                        ("u4d", [H, 2])):
        consts[name] = const.tile(shape, BF16, tag=name, name=f"c_{name}")

    z0_all = z_pool.tile([H, BS], BF16)   # relu(pooled @ W3U0 + c0') all rows

    # pipeline state
    h1_ps = [None] * PAIRS
    h1_sb = [None] * PAIRS
    h2_ps = [None] * (2 * PAIRS)
    h2_sb = [None] * (2 * PAIRS)
    z0_ps = [None] * SUP

    # supertile 0: PE transpose x_sb -> xt_sb[0], quarter-pipelined
    xt_sb[0] = xt_pool.tile([128, ROW], BF16, tag="xt_sb", name="xt_sb0")
    xt_ps0 = ps_h1.tile([128, ROW], F32, tag="h1_ps", name="xt_ps0")
    for m in range(4):
        nc.tensor.matmul(
            xt_ps0[:, 128 * m:128 * (m + 1)], x_sb[:, 128 * m:128 * (m + 1)],
            ident[:], start=True, stop=True)
    nc.vector.tensor_copy(xt_sb[0][:], xt_ps0[:])

    emit_transposes(1)

    for i in range(PAIRS + 4):
        # stage A: L01 pair -> h1_ps [100, 1024] (2 banks)
        if i < PAIRS:
            s = i // PAIRS_PER_SUP
            h1_ps[i] = ps_h1.tile([H, 2 * ROW], F32, tag="h1_ps", name=f"h1_ps{i}")
            for half in range(2):
                t = 2 * (i % PAIRS_PER_SUP) + half
                nc.tensor.matmul(
                    h1_ps[i][:, half * ROW:(half + 1) * ROW],
                    w01_slice(t), xt_sb[s][:],
                    start=True, stop=True,
                )

        # stage B: ep1 (bias+relu, PSUM->SBUF bf16), 2t wide — always ACT
        j = i - 1
        if 0 <= j < PAIRS:
            h1_sb[j] = h1s_pool.tile([H, 2 * ROW], BF16, tag="h1_sb", name=f"h1_sb{j}")
            nc.scalar.activation(h1_sb[j][:], h1_ps[j][:], AF.Relu, bias=b1p)

        # stage C: L2 pair -> two 1t h2_ps tiles
        k = i - 2
        if 0 <= k < PAIRS:
            for half in range(2):
                tt = 2 * k + half
                h2_ps[tt] = ps_h2.tile([H, ROW], F32, tag="h2_ps", name=f"h2_ps{tt}")
                nc.tensor.matmul(
                    h2_ps[tt][:], consts["w2"][:],
                    h1_sb[k][:, half * ROW:(half + 1) * ROW],
                    start=True, stop=True,
                )

        # stage D: ep2 x2 — DVE, except every 7th pair's first half on ACT
        l = i - 3
        if 0 <= l < PAIRS:
            for half in range(2):
                tt = 2 * l + half
                h2_sb[tt] = h2s_pool.tile([H, ROW], BF16, tag="h2_sb", name=f"h2_sb{tt}")
                if half == 0 and l % 4 == 1:
                    nc.scalar.activation(
                        h2_sb[tt][:], h2_ps[tt][:], AF.Relu, bias=b2)
                else:
                    nc.vector.tensor_scalar(
                        h2_sb[tt][:], h2_ps[tt][:], b2, 0.0, ALU.add, ALU.max)

        # stage E: pool-accumulate pair into z0_ps [100, ROW] (m-fold later)
        m = i - 4
        if 0 <= m < PAIRS:
            s = m // PAIRS_PER_SUP
            mi = m % PAIRS_PER_SUP
            if mi == 0:
                z0_ps[s] = ps_z.tile([H, ROW], F32, tag="z0_ps", name=f"z0_ps{s}")
            for half in range(2):
                tt = 2 * m + half
                nc.tensor.matmul(
                    z0_ps[s][:], consts["w3u0"][:], h2_sb[tt][:],
                    start=(mi == 0 and half == 0),
                    stop=(mi == PAIRS_PER_SUP - 1 and half == 1),
                )
            if mi == PAIRS_PER_SUP - 1:
                # fold the 4 m-blocks, then bias+relu into z0_all slice
                z0_raw = h1s_pool.tile(
                    [H, ROW], F32, tag="z0_raw", name=f"z0_raw{s}")
                nc.scalar.copy(z0_raw[:], z0_ps[s][:])
                # last supertile: fold on DVE (idle by then, shortens the
                # head-transition critical path); otherwise GpSimd
                feng = nc.vector if s == SUP - 1 else nc.gpsimd
                zf0 = h1s_pool.tile([H, SB], F32, tag="zf0", name=f"zf0_{s}")
                feng.tensor_tensor(
                    zf0[:], z0_raw[:, 0:SB], z0_raw[:, SB:2 * SB], ALU.add)
                zf1 = h1s_pool.tile([H, SB], F32, tag="zf1", name=f"zf1_{s}")
                feng.tensor_tensor(
                    zf1[:], z0_raw[:, 2 * SB:3 * SB], z0_raw[:, 3 * SB:4 * SB],
                    ALU.add)
                zf = h1s_pool.tile([H, SB], F32, tag="zf", name=f"zf_{s}")
                feng.tensor_tensor(zf[:], zf0[:], zf1[:], ALU.add)
                nc.scalar.activation(
                    z0_all[:, s * SB:(s + 1) * SB], zf[:], AF.Relu, bias=c0p)
                if s + 2 < SUP:
                    emit_transposes(s + 2)

    # --- event head on all 512 batch rows ---
    for name in ("u1", "u2", "u3", "u4d"):
        nc.sync.dma_start(consts[name][:], d[name].ap())
    zin = z0_all
    for li, (u, c) in enumerate((("u1", c1), ("u2", c2), ("u3", c3))):
        zh = h1s_pool.tile([H, BS], BF16, tag="h1_sb", name=f"zh{li}")
        for hh in range(2):
            zh_ps = ps_h2.tile([H, BS // 2], F32, tag="h2_ps",
                               name=f"zh_ps{li}_{hh}")
            nc.tensor.matmul(
                zh_ps[:], consts[u][:],
                zin[:, hh * (BS // 2):(hh + 1) * (BS // 2)],
                start=True, stop=True)
            if hh == 0:
                nc.scalar.activation(
                    zh[:, hh * (BS // 2):(hh + 1) * (BS // 2)], zh_ps[:],
                    AF.Relu, bias=c)
            else:
                nc.vector.tensor_scalar(
                    zh[:, hh * (BS // 2):(hh + 1) * (BS // 2)], zh_ps[:],
                    c, 0.0, ALU.add, ALU.max)
        zin = zh

    d_ps = ps_h2.tile([2, BS], F32, tag="h2_ps")
    nc.tensor.matmul(d_ps[:], consts["u4d"][:], zin[:], start=True, stop=True)
    probs = h1s_pool.tile([2, BS], F32, tag="probs")
    nc.scalar.activation(probs[:], d_ps[:], AF.Sigmoid, bias=c4d)
    nc.sync.dma_start(d["y"].ap(), probs[:])


def _build():
    nc = bacc.Bacc("TRN2", target_bir_lowering=False, debug=False)
    d = {}
    d["x"] = nc.dram_tensor("x", [BS, ROW], BF16, kind="ExternalInput")
    d["w01"] = nc.dram_tensor("w01", [128, NT * H], BF16, kind="ExternalInput")
    for name, shape in (("w2", [H, H]), ("w3u0", [H, H]), ("u1", [H, H]),
                        ("u2", [H, H]), ("u3", [H, H]), ("u4d", [H, 2])):
        d[name] = nc.dram_tensor(name, shape, BF16, kind="ExternalInput")
    d["bias"] = nc.dram_tensor("bias", [128, 8], F32, kind="ExternalInput")
    d["ident"] = nc.dram_tensor("ident", [128, 128], BF16, kind="ExternalInput")
    d["y"] = nc.dram_tensor("y", [2, BS], F32, kind="ExternalOutput")

    with tile.TileContext(nc) as tc:
        _body(tc, d)
    nc.compile()
    return nc


_NC = None


def _get_nc():
    global _NC
    if _NC is None:
        _NC = _build()
    return _NC


def _prep_inputs(inputs):
    f32 = np.float32
    bf16 = ml_dtypes.bfloat16
    W0, b0 = np.asarray(inputs["W0"], f32), np.asarray(inputs["b0"], f32)
    W1, b1 = np.asarray(inputs["W1"], f32), np.asarray(inputs["b1"], f32)
    W2, b2 = np.asarray(inputs["W2"], f32), np.asarray(inputs["b2"], f32)
    W3, b3 = np.asarray(inputs["W3"], f32), np.asarray(inputs["b3"], f32)
    U0, c0 = np.asarray(inputs["U0"], f32), np.asarray(inputs["c0"], f32)
    U1, c1 = np.asarray(inputs["U1"], f32), np.asarray(inputs["c1"], f32)
    U2, c2 = np.asarray(inputs["U2"], f32), np.asarray(inputs["c2"], f32)
    U3, c3 = np.asarray(inputs["U3"], f32), np.asarray(inputs["c3"], f32)
    U4, c4 = np.asarray(inputs["U4"], f32), np.asarray(inputs["c4"], f32)

    W01 = W0 @ W1
    b1p = b0 @ W1 + b1
    W3U0 = W3 @ U0
    c0p = np.float32(P) * (b3 @ U0) + c0
    u4diff = U4[:, 0] - U4[:, 1]
    u4d = np.stack([u4diff, -u4diff], axis=1)
    c4d = np.array([c4[0] - c4[1], c4[1] - c4[0]], f32)

    w01p = np.zeros((128, NT * H), f32)
    for t in range(NT):
        w01p[4 * t:4 * t + 4, H * t:H * (t + 1)] = W01

    bias = np.zeros((128, 8), f32)
    bias[:H, 0] = b1p
    bias[:H, 1] = b2
    bias[:H, 2] = c0p
    bias[:H, 3] = c1
    bias[:H, 4] = c2
    bias[:H, 5] = c3
    bias[0:2, 6] = c4d

    shared = {
        "w01": w01p.astype(bf16),
        "w2": W2.astype(bf16),
        "w3u0": W3U0.astype(bf16),
        "u1": U1.astype(bf16),
        "u2": U2.astype(bf16),
        "u3": U3.astype(bf16),
        "u4d": u4d.astype(bf16),
        "bias": bias,
        "ident": np.eye(128, dtype=f32).astype(bf16),
    }
    x = np.asarray(inputs["x"], f32).reshape(B, ROW).astype(bf16)
    in_maps = []
    for k in range(NCORES):
        m = dict(shared)
        m["x"] = np.ascontiguousarray(x[k * BS:(k + 1) * BS])
        in_maps.append(m)
    return in_maps


def kernel(**inputs):
    nc = _get_nc()
    in_maps = _prep_inputs(inputs)
    res = run_bass_kernel_spmd(nc, in_maps, list(range(NCORES)))
    out = np.empty((B, 2), np.float32)
    for k in range(NCORES):
        y = np.asarray(res.results[k]["y"])  # [2, BS]
        out[k * BS:(k + 1) * BS, 0] = y[0]
        out[k * BS:(k + 1) * BS, 1] = y[1]
    return out


# revision 43
# speedup vs baseline: 1.0231x; 1.0231x over previous
"""ParticleFlowNetwork TRN2 Bass kernel (v2 — software-pipelined).

Network (B=4096, P=128, IN=4, H=100):
    h = x @ W0 + b0            (no ReLU)
    h = relu(h @ W1 + b1); h = relu(h @ W2 + b2); h = h @ W3 + b3
    lat = sum over P; 5-layer event head; softmax(2)

Host-side algebraic folds (linear, exact):
  * W01 = W0 @ W1, b1' = b0 @ W1 + b1      (no ReLU after layer 0)
  * pooling moved before W3; W3U0 = W3 @ U0, c0' = P*(b3 @ U0) + c0
  * 2-class softmax == sigmoid of +/- logit difference.

Device structure per core (512 batch rows, 4 supertiles x 128 rows):
  * x pre-cast to bf16 on host. Supertile 0 transposed via PE matmul
    (x tile stationary x identity) while the weight DMAs stream;
    supertiles 1-3 via DMA xbar transpose (DRAM->SBUF, off PE).
  * all activations transposed [hidden, tokens]; bf16 matmul operands,
    fp32 PSUM. Per supertile, 32 particle-tiles t of [100, 512].
  * 5-stage software pipeline over 64 token-pairs (L01 pair -> ep1 ->
    L2 pair -> ep2 x2 -> pool pair, with 1-iteration offsets) so PE
    matmuls stream back-to-back at ~215 ns/512-col.
  * engine DEDICATION avoids FIFO head-of-line blocking: ACT runs all
    2t-wide ep1 (bias+ReLU, PSUM->SBUF, [100,1024]); DVE runs the 1t
    ep2s except every 6th pair's first half on ACT for load balance.
  * pooling = PSUM-accumulated matmuls (W3U0 as lhsT) over all 32
    particle tiles; the 4 column-block partial sums are folded by
    GpSimd (DVE for the last supertile) after an ACT PSUM->SBUF copy.
  * event head pipelined in column halves; softmax via one Sigmoid on
    [2, 512]; output stays [2, 512] per core (host reassembles).

Measured on 8 axon trn2 cores: ~111.2 us NEFF exec, rel err 1.4e-4.
PSUM budget: h1 [100,1024]x2 + h2 [100,512]x3 + z0 [100,512]x1 = 8 banks.
"""

from contextlib import ExitStack

import numpy as np
import ml_dtypes

import concourse.bass as bass
import concourse.tile as tile
from concourse import bacc, mybir
from concourse._compat import with_exitstack
from concourse.bass_utils import run_bass_kernel_spmd

F32 = mybir.dt.float32
BF16 = mybir.dt.bfloat16
AF = mybir.ActivationFunctionType
ALU = mybir.AluOpType

B, P, IN, H = 4096, 128, 4, 100
NCORES = 8
BS = B // NCORES          # 512 batch rows per core
SUP = 4                   # supertiles per core
SB = BS // SUP            # 128 batch rows per supertile
ROW = P * IN              # 512 bf16 per batch row
NT = P // 4               # 32 particle-tiles (t) per supertile
PAIRS_PER_SUP = NT // 2   # 16
PAIRS = SUP * PAIRS_PER_SUP  # 64 global pair iterations
W01_CHUNK = 4             # t-slices per w01 DMA chunk


@with_exitstack
def _body(ctx: ExitStack, tc: "tile.TileContext", d):
    nc = tc.nc

    const = ctx.enter_context(tc.tile_pool(name="const", bufs=1))
    xt_pool = ctx.enter_context(tc.tile_pool(name="xt", bufs=2))
    h1s_pool = ctx.enter_context(tc.tile_pool(name="h1s", bufs=3))
    h2s_pool = ctx.enter_context(tc.tile_pool(name="h2s", bufs=5))
    z_pool = ctx.enter_context(tc.tile_pool(name="z", bufs=1))
    ps_h1 = ctx.enter_context(tc.tile_pool(name="ps_h1", bufs=2, space="PSUM"))
    ps_h2 = ctx.enter_context(tc.tile_pool(name="ps_h2", bufs=3, space="PSUM"))
    ps_z = ctx.enter_context(tc.tile_pool(name="ps_z", bufs=1, space="PSUM"))

    # pipeline state (xt declared before first transpose emission)
    xt_sb = [None] * SUP

    def emit_transposes(s, split=False):
        # x rows 128s..128s+128, bf16 [128, ROW] -> xT [128, ROW] via xbar
        xt_sb[s] = xt_pool.tile([128, ROW], BF16, tag="xt_sb", name=f"xt_sb{s}")
        for m in range(4):
            eng = nc.scalar if (split and m % 2) else nc.sync
            eng.dma_start_transpose(
                xt_sb[s][:, 128 * m:128 * (m + 1)],
                d["x"].ap()[s * SB:(s + 1) * SB, 128 * m:128 * (m + 1)],
            )

    # supertile 0 via PE transpose (PE idle at startup; xbar path would
    # serialize against the startup weight DMAs on the HWDGE engines)
    x_sb = const.tile([128, ROW], BF16, tag="x_sb")
    nc.sync.dma_start(x_sb[:], d["x"].ap()[0:SB, :])
    ident = const.tile([128, 128], BF16, tag="ident")
    nc.sync.dma_start(ident[:], d["ident"].ap())
    bias = const.tile([128, 8], F32, tag="bias")
    nc.scalar.dma_start(bias[:], d["bias"].ap())
    b1p, b2 = bias[:H, 0:1], bias[:H, 1:2]
    c0p, c1, c2, c3 = bias[:H, 2:3], bias[:H, 3:4], bias[:H, 4:5], bias[:H, 5:6]
    c4d = bias[0:2, 6:7]

    # dummy sigmoid so walrus loads the sigmoid table-set (contains relu
    # too) once at startup instead of a second ACT_TABLE_LOAD at the tail
    dummy = const.tile([1, 1], F32, tag="dummy")
    nc.scalar.activation(dummy[:], bias[0:1, 7:8], AF.Sigmoid)

    w01c = []
    w01_dmas = []
    for j in range(NT // W01_CHUNK):
        w = const.tile([128, W01_CHUNK * H], BF16, tag=f"w01_{j}", name=f"w01_{j}")
        w01c.append(w)

    def emit_w01(j, eng=None):
        (eng or nc.sync).dma_start(
            w01c[j][:],
            d["w01"].ap()[:, j * W01_CHUNK * H:(j + 1) * W01_CHUNK * H])

    def w01_slice(t):
        return w01c[t // W01_CHUNK][:, (t % W01_CHUNK) * H:(t % W01_CHUNK + 1) * H]

    emit_w01(0, eng=nc.scalar)
    consts = {}
    for name, shape in (("w2", [H, H]), ("w3u0", [H, H])):
        tl = const.tile(shape, BF16, tag=name, name=f"c_{name}")
        nc.sync.dma_start(tl[:], d[name].ap())
        consts[name] = tl
    for j in range(1, NT // W01_CHUNK):
        emit_w01(j)
    # head weights are only needed at the very end; declared here, DMA'd
    # after the main loop so they don't clog the startup DMA queue
    for name, shape in (("u1", [H, H]), ("u2", [H, H]), ("u3", [H, H]),
                        ("u4d", [H, 2])):
        consts[name] = const.tile(shape, BF16, tag=name, name=f"c_{name}")

    z0_all = z_pool.tile([H, BS], BF16)   # relu(pooled @ W3U0 + c0') all rows

    # pipeline state
    h1_ps = [None] * PAIRS
    h1_sb = [None] * PAIRS
    h2_ps = [None] * (2 * PAIRS)
    h2_sb = [None] * (2 * PAIRS)
    z0_ps = [None] * SUP

    # supertile 0: PE transpose x_sb -> xt_sb[0]
    xt_sb[0] = xt_pool.tile([128, ROW], BF16, tag="xt_sb", name="xt_sb0")
    xt_ps0 = ps_h1.tile([128, ROW], F32, tag="h1_ps", name="xt_ps0")
    for m in range(4):
        nc.tensor.matmul(
            xt_ps0[:, 128 * m:128 * (m + 1)], x_sb[:, 128 * m:128 * (m + 1)],
            ident[:], start=True, stop=True)
    nc.vector.tensor_copy(xt_sb[0][:], xt_ps0[:])

    emit_transposes(1)

    for i in range(PAIRS + 4):
        # stage A: L01 pair -> h1_ps [100, 1024] (2 banks)
        if i < PAIRS:
            s = i // PAIRS_PER_SUP
            h1_ps[i] = ps_h1.tile([H, 2 * ROW], F32, tag="h1_ps", name=f"h1_ps{i}")
            for half in range(2):
                t = 2 * (i % PAIRS_PER_SUP) + half
                nc.tensor.matmul(
                    h1_ps[i][:, half * ROW:(half + 1) * ROW],
                    w01_slice(t), xt_sb[s][:],
                    start=True, stop=True,
                )

        # stage B: ep1 (bias+relu, PSUM->SBUF bf16), 2t wide — always ACT
        j = i - 1
        if 0 <= j < PAIRS:
            h1_sb[j] = h1s_pool.tile([H, 2 * ROW], BF16, tag="h1_sb", name=f"h1_sb{j}")
            nc.scalar.activation(h1_sb[j][:], h1_ps[j][:], AF.Relu, bias=b1p)

        # stage C: L2 pair -> two 1t h2_ps tiles
        k = i - 2
        if 0 <= k < PAIRS:
            for half in range(2):
                tt = 2 * k + half
                h2_ps[tt] = ps_h2.tile([H, ROW], F32, tag="h2_ps", name=f"h2_ps{tt}")
                nc.tensor.matmul(
                    h2_ps[tt][:], consts["w2"][:],
                    h1_sb[k][:, half * ROW:(half + 1) * ROW],
                    start=True, stop=True,
                )

        # stage D: ep2 x2 — DVE, except every 7th pair's first half on ACT
        l = i - 3
        if 0 <= l < PAIRS:
            for half in range(2):
                tt = 2 * l + half
                h2_sb[tt] = h2s_pool.tile([H, ROW], BF16, tag="h2_sb", name=f"h2_sb{tt}")
                if half == 0 and l % 4 == 1:
                    nc.scalar.activation(
                        h2_sb[tt][:], h2_ps[tt][:], AF.Relu, bias=b2)
                else:
                    nc.vector.tensor_scalar(
                        h2_sb[tt][:], h2_ps[tt][:], b2, 0.0, ALU.add, ALU.max)

        # stage E: pool-accumulate pair into z0_ps [100, ROW] (m-fold later)
        m = i - 4
        if 0 <= m < PAIRS:
            s = m // PAIRS_PER_SUP
            mi = m % PAIRS_PER_SUP
            if mi == 0:
                z0_ps[s] = ps_z.tile([H, ROW], F32, tag="z0_ps", name=f"z0_ps{s}")
            for half in range(2):
                tt = 2 * m + half
                nc.tensor.matmul(
                    z0_ps[s][:], consts["w3u0"][:], h2_sb[tt][:],
                    start=(mi == 0 and half == 0),
                    stop=(mi == PAIRS_PER_SUP - 1 and half == 1),
                )
            if mi == PAIRS_PER_SUP - 1:
                # fold the 4 m-blocks, then bias+relu into z0_all slice
                z0_raw = h1s_pool.tile(
                    [H, ROW], F32, tag="z0_raw", name=f"z0_raw{s}")
                nc.scalar.copy(z0_raw[:], z0_ps[s][:])
                # last supertile: fold on DVE (idle by then, shortens the
                # head-transition critical path); otherwise GpSimd
                feng = nc.vector if s == SUP - 1 else nc.gpsimd
                zf0 = h1s_pool.tile([H, SB], F32, tag="zf0", name=f"zf0_{s}")
                feng.tensor_tensor(
                    zf0[:], z0_raw[:, 0:SB], z0_raw[:, SB:2 * SB], ALU.add)
                zf1 = h1s_pool.tile([H, SB], F32, tag="zf1", name=f"zf1_{s}")
                feng.tensor_tensor(
                    zf1[:], z0_raw[:, 2 * SB:3 * SB], z0_raw[:, 3 * SB:4 * SB],
                    ALU.add)
                zf = h1s_pool.tile([H, SB], F32, tag="zf", name=f"zf_{s}")
                feng.tensor_tensor(zf[:], zf0[:], zf1[:], ALU.add)
                nc.scalar.activation(
                    z0_all[:, s * SB:(s + 1) * SB], zf[:], AF.Relu, bias=c0p)
                if s + 2 < SUP:
                    emit_transposes(s + 2)

    # --- event head on all 512 batch rows ---
    for name in ("u1", "u2", "u3", "u4d"):
        nc.sync.dma_start(consts[name][:], d[name].ap())
    zin = z0_all
    for li, (u, c) in enumerate((("u1", c1), ("u2", c2), ("u3", c3))):
        zh = h1s_pool.tile([H, BS], BF16, tag="h1_sb", name=f"zh{li}")
        for hh in range(2):
            zh_ps = ps_h2.tile([H, BS // 2], F32, tag="h2_ps",
                               name=f"zh_ps{li}_{hh}")
            nc.tensor.matmul(
                zh_ps[:], consts[u][:],
                zin[:, hh * (BS // 2):(hh + 1) * (BS // 2)],
                start=True, stop=True)
            nc.scalar.activation(
                zh[:, hh * (BS // 2):(hh + 1) * (BS // 2)], zh_ps[:],
                AF.Relu, bias=c)
        zin = zh

    d_ps = ps_h2.tile([2, BS], F32, tag="h2_ps")
    nc.tensor.matmul(d_ps[:], consts["u4d"][:], zin[:], start=True, stop=True)
    probs = h1s_pool.tile([2, BS], F32, tag="probs")
    nc.scalar.activation(probs[:], d_ps[:], AF.Sigmoid, bias=c4d)
    nc.sync.dma_start(d["y"].ap(), probs[:])


def _build():
    nc = bacc.Bacc("TRN2", target_bir_lowering=False, debug=False)
    d = {}
    d["x"] = nc.dram_tensor("x", [BS, ROW], BF16, kind="ExternalInput")
    d["w01"] = nc.dram_tensor("w01", [128, NT * H], BF16, kind="ExternalInput")
    for name, shape in (("w2", [H, H]), ("w3u0", [H, H]), ("u1", [H, H]),
                        ("u2", [H, H]), ("u3", [H, H]), ("u4d", [H, 2])):
        d[name] = nc.dram_tensor(name, shape, BF16, kind="ExternalInput")
    d["bias"] = nc.dram_tensor("bias", [128, 8], F32, kind="ExternalInput")
    d["ident"] = nc.dram_tensor("ident", [128, 128], BF16, kind="ExternalInput")
    d["y"] = nc.dram_tensor("y", [2, BS], F32, kind="ExternalOutput")

    with tile.TileContext(nc) as tc:
        _body(tc, d)
    nc.compile()
    return nc


_NC = None


def _get_nc():
    global _NC
    if _NC is None:
        _NC = _build()
    return _NC


def _prep_inputs(inputs):
    f32 = np.float32
    bf16 = ml_dtypes.bfloat16
    W0, b0 = np.asarray(inputs["W0"], f32), np.asarray(inputs["b0"], f32)
    W1, b1 = np.asarray(inputs["W1"], f32), np.asarray(inputs["b1"], f32)
    W2, b2 = np.asarray(inputs["W2"], f32), np.asarray(inputs["b2"], f32)
    W3, b3 = np.asarray(inputs["W3"], f32), np.asarray(inputs["b3"], f32)
    U0, c0 = np.asarray(inputs["U0"], f32), np.asarray(inputs["c0"], f32)
    U1, c1 = np.asarray(inputs["U1"], f32), np.asarray(inputs["c1"], f32)
    U2, c2 = np.asarray(inputs["U2"], f32), np.asarray(inputs["c2"], f32)
    U3, c3 = np.asarray(inputs["U3"], f32), np.asarray(inputs["c3"], f32)
    U4, c4 = np.asarray(inputs["U4"], f32), np.asarray(inputs["c4"], f32)

    W01 = W0 @ W1
    b1p = b0 @ W1 + b1
    W3U0 = W3 @ U0
    c0p = np.float32(P) * (b3 @ U0) + c0
    u4diff = U4[:, 0] - U4[:, 1]
    u4d = np.stack([u4diff, -u4diff], axis=1)
    c4d = np.array([c4[0] - c4[1], c4[1] - c4[0]], f32)

    w01p = np.zeros((128, NT * H), f32)
    for t in range(NT):
        w01p[4 * t:4 * t + 4, H * t:H * (t + 1)] = W01

    bias = np.zeros((128, 8), f32)
    bias[:H, 0] = b1p
    bias[:H, 1] = b2
    bias[:H, 2] = c0p
    bias[:H, 3] = c1
    bias[:H, 4] = c2
    bias[:H, 5] = c3
    bias[0:2, 6] = c4d

    shared = {
        "w01": w01p.astype(bf16),
        "w2": W2.astype(bf16),
        "w3u0": W3U0.astype(bf16),
        "u1": U1.astype(bf16),
        "u2": U2.astype(bf16),
        "u3": U3.astype(bf16),
        "u4d": u4d.astype(bf16),
        "bias": bias,
        "ident": np.eye(128, dtype=f32).astype(bf16),
    }
    x = np.asarray(inputs["x"], f32).reshape(B, ROW).astype(bf16)
    in_maps = []
    for k in range(NCORES):
        m = dict(shared)
        m["x"] = np.ascontiguousarray(x[k * BS:(k + 1) * BS])
        in_maps.append(m)
    return in_maps


def kernel(**inputs):
    nc = _get_nc()
    in_maps = _prep_inputs(inputs)
    res = run_bass_kernel_spmd(nc, in_maps, list(range(NCORES)))
    out = np.empty((B, 2), np.float32)
    for k in range(NCORES):
        y = np.asarray(res.results[k]["y"])  # [2, BS]
        out[k * BS:(k + 1) * BS, 0] = y[0]
        out[k * BS:(k + 1) * BS, 1] = y[1]
    return out
